# revision 10
# baseline (speedup 1.0000x reference)
"""CoAttention ImageDNS kernel for Trainium2 (8 NeuronCores, Bass/Tile).

Math: the reference computes two additive-attention blocks. In both, the
softmax'd score is  score[b, q, k] = f(q-side)[b, q] + g(k-side)[b, k] + c,
and softmax over k is invariant to the q-dependent (and constant) terms, so
the attention weights are independent of the query index:

  visual_att[b, s, :]  = softmax_r( wB . tanh(W_i1 @ img[b, r]) )
  textual_att[b, i, :] = softmax_j( wD . tanh(W_d2 @ dns[b, j]) )

Hence both outputs are per-batch rank-1 broadcasts:

  att_img_features[b, s, :] = visual_att[b]  @ img[b]   (same for all s)
  att_dns_features[b, i, :] = textual_att[b] @ dns[b]   (same for all i)

W_d1/b_d1/w_att1[:H]/b_att1/W_i2/b_i2/w_att2[:H]/b_att2 cancel entirely.

Sharding: pure data-parallel over batch, 4 batches per core, no collectives.
The device computes the per-batch [H] attention outputs; the host broadcasts
them over the (identical) S query rows, so the kernel writes only B*H values
instead of B*S*H.

Layout: projections run transposed (weights stationary, activations moving):
proj^T[o, r] accumulates per 128-wide o-chunk over the h-chunks, so the
score reduction over o is 8 tiny PE matmuls against the w-column and the
scores land in ROW layout on partition 0.  From there: exp (+softmax sum via
accum) on Scalar, partition-broadcast of the weight row on GpSimd, stage-2
weighted row sums on Vector (stt accum over the same transposed activation
tiles the projections use - the natural-layout activations are never
loaded).  The PE stream is pure projection + score matmuls; each group's
softmax/stage-2 tail is emitted one group later so it hides under the next
group's projections.  HBM in is ~10MB/core.

Work per core: PE ~86us of matmul streaming (the bf16 roofline for the
32*708*1024*1024-MAC projection at 128x128 MACs/cycle x 2.4GHz), Vector
~32us, Scalar ~22us, DMA ~28us.
"""

import sys
import numpy as np
import ml_dtypes

_BF16 = ml_dtypes.bfloat16

for _p in ("/opt/trn_rl_repo", "/root/.axon_site/_ro/trn_rl_repo"):
    if _p not in sys.path:
        sys.path.append(_p)

B, S, R, H = 32, 512, 196, 1024
NCORES = 8
BLOC = B // NCORES          # batches per core
HC = H // 128               # contraction chunks of 128
OC = H // 128               # projection output chunks of 128
NI = BLOC * R               # img rows, all batches packed (784)
ND = BLOC * S               # dns rows, all batches packed (2048)
GI = 2 * R                  # img group rows (2 batches)

_CACHE = {}


def build_nc():
    from concourse import bacc, mybir
    from concourse import tile

    f32, f16 = mybir.dt.float32, mybir.dt.bfloat16
    Act = mybir.ActivationFunctionType
    Alu = mybir.AluOpType

    nc = bacc.Bacc("TRN2", target_bir_lowering=False, debug=False)

    xt_img = nc.dram_tensor("xt_img", [HC, 128, NI], f16, kind="ExternalInput")
    xt_dns = nc.dram_tensor("xt_dns", [HC, 128, ND], f16, kind="ExternalInput")
    # oc-major weight layout: [oc, hc, 128(h), 128(o)] so one 0.25MB DMA
    # delivers everything one proj accumulation group needs
    wt_i1 = nc.dram_tensor("wt_i1", [OC, HC, 128, 128], f16, kind="ExternalInput")
    wt_d2 = nc.dram_tensor("wt_d2", [OC, HC, 128, 128], f16, kind="ExternalInput")
    wcol_b = nc.dram_tensor("wcol_b", [128, OC], f16, kind="ExternalInput")
    wcol_d = nc.dram_tensor("wcol_d", [128, OC], f16, kind="ExternalInput")
    # [p, b, hc] layout: element h of batch b lives at [h % 128, b, h // 128]
    out_dns = nc.dram_tensor("out_dns", [128, BLOC, HC], f32, kind="ExternalOutput")
    out_img = nc.dram_tensor("out_img", [128, BLOC, HC], f32, kind="ExternalOutput")

    with tile.TileContext(nc) as tc:
        with (
            tc.tile_pool(name="const", bufs=1) as cpool,
            tc.tile_pool(name="work", bufs=3) as wpool,
            tc.tile_pool(name="small", bufs=8) as spool,
            tc.tile_pool(name="ppd", bufs=2, space="PSUM") as ppd,
            tc.tile_pool(name="ppi", bufs=2, space="PSUM") as ppi,
            tc.tile_pool(name="psr", bufs=2, space="PSUM") as psr,
        ):
            xt_i = cpool.tile([128, HC * NI], f16, name="xt_img_sb")
            xt_d = cpool.tile([128, HC * ND], f16, name="xt_dns_sb")
            wt_sb = {"img": cpool.tile([128, OC * HC * 128], f16, name="wt_i1_sb"),
                     "dns": cpool.tile([128, OC * HC * 128], f16, name="wt_d2_sb")}
            wc_sb = {"img": cpool.tile([128, OC], f16, name="wcol_b_sb"),
                     "dns": cpool.tile([128, OC], f16, name="wcol_d_sb")}
            att_sb = {s: cpool.tile([128, BLOC * HC], f32, name=f"att_{s}_sb")
                      for s in ("img", "dns")}

            wt_dram = {"img": wt_i1, "dns": wt_d2}
            wc_dram = {"img": wcol_b, "dns": wcol_d}
            xt_sb = {"img": xt_i, "dns": xt_d}
            xt_dram = {"img": xt_img, "dns": xt_dns}
            n_all = {"img": NI, "dns": ND}
            out_d = {"img": out_img, "dns": out_dns}

            def load_wt_oc(side, oc):
                w = wt_sb[side]
                nc.sync.dma_start(
                    out=w[:, oc * HC * 128:(oc + 1) * HC * 128]
                    .rearrange("p (hc m) -> p hc m", hc=HC),
                    in_=wt_dram[side][oc].rearrange("hc p m -> p hc m"))

            def load_xt(side, c0, c1):
                nc.sync.dma_start(
                    out=xt_sb[side].rearrange("p (hc m) -> p hc m", hc=HC)
                    [:, :, c0:c1],
                    in_=xt_dram[side][:, :, c0:c1].rearrange("hc p m -> p hc m"))

            wt3 = {s: wt_sb[s].rearrange("p (oc hc m) -> p oc hc m", oc=OC, hc=HC)
                   for s in ("img", "dns")}
            xt3 = {s: xt_sb[s].rearrange("p (hc m) -> p hc m", hc=HC)
                   for s in ("img", "dns")}

            # groups: (side, row0, row1, [(batch, off_in_group), ...])
            # dns first (per batch), img last in 2-batch groups (small tail)
            groups = [("dns", b * S, (b + 1) * S, [(b, 0)]) for b in range(BLOC)]
            groups += [("img", g * GI, (g + 1) * GI, [(2 * g, 0), (2 * g + 1, R)])
                       for g in range(2)]

            def emit_loads(gi):
                side, g0, g1, _ = groups[gi]
                if gi == 0:
                    load_wt_oc("dns", 0)
                    # per-hc xt chunks: the first proj matmul only waits for
                    # wt oc0 + one 128KB chunk instead of the whole batch
                    for hc in range(HC):
                        nc.sync.dma_start(
                            out=xt3["dns"][:, hc, 0:S],
                            in_=xt_dram["dns"][hc, :, 0:S])
                    for oc in range(1, OC):
                        load_wt_oc("dns", oc)
                    nc.sync.dma_start(out=wc_sb["dns"][:, :], in_=wc_dram["dns"][:, :])
                elif side == "dns":
                    load_xt("dns", g0, g1)
                elif side == "img" and g0 == 0:
                    for oc in range(OC):
                        load_wt_oc("img", oc)
                    load_xt("img", 0, GI)
                    nc.sync.dma_start(out=wc_sb["img"][:, :], in_=wc_dram["img"][:, :])
                else:
                    load_xt("img", g0, g1)

            def emit_group(gi, prev_score7, prev_tail):
                side, g0, g1, _ = groups[gi]
                ng = g1 - g0
                pp = ppd if side == "dns" else ppi
                srow = psr.tile([1, ng], f32, name=f"srow_{gi}", tag=f"srow_{side}")
                ths = []

                def score_mm(oc):
                    nc.tensor.matmul(
                        srow[0:1, 0:ng], lhsT=wc_sb[side][:, oc:oc + 1],
                        rhs=ths[oc][:, 0:ng],
                        start=(oc == 0), stop=(oc == OC - 1))

                emit_loads(gi)
                for oc in range(OC):
                    ps = pp.tile([128, ng], f32, name=f"proj_{gi}_{oc}",
                                 tag=f"pp_{side}")
                    for hc in range(HC):
                        nc.tensor.matmul(
                            ps[:, 0:ng],
                            lhsT=wt3[side][:, oc, hc, :],
                            rhs=xt3[side][:, hc, g0:g1],
                            start=(hc == 0), stop=(hc == HC - 1))
                    th = wpool.tile([128, ng], f16, name=f"th_{gi}_{oc}",
                                    tag=f"th_{side}", bufs=3)
                    nc.scalar.activation(th[:, 0:ng], ps[:, 0:ng], Act.Tanh)
                    ths.append(th)
                    if oc == 0 and prev_score7 is not None:
                        prev_score7()
                    if oc == 1 and prev_tail is not None:
                        prev_tail()
                    if oc > 0:
                        score_mm(oc - 1)
                return (lambda: score_mm(OC - 1)), (lambda: emit_tail(gi, srow))

            def emit_tail(gi, srow):
                side, g0, g1, batches = groups[gi]
                ng = g1 - g0
                nr = S if side == "dns" else R
                arow = spool.tile([1, ng], f16, name=f"arow_{gi}", tag=f"arow_{side}",
                                  bufs=2)
                a_b = wpool.tile([128, ng], f16, name=f"ab_{gi}", tag=f"ab_{side}",
                                 bufs=2)
                for b, off in batches:
                    ssum = spool.tile([1, 1], f32, name=f"ss_{gi}_{b}", tag="ssum",
                                      bufs=4)
                    nc.scalar.activation(arow[0:1, off:off + nr],
                                         srow[0:1, off:off + nr], Act.Exp,
                                         accum_out=ssum[0:1, 0:1])
                    rr = spool.tile([1, 1], f32, name=f"rr_{gi}_{b}", tag="rr",
                                    bufs=4)
                    nc.vector.reciprocal(rr[0:1, 0:1], ssum[0:1, 0:1])
                    rb = spool.tile([128, 1], f32, name=f"rb_{gi}_{b}", tag="rb",
                                    bufs=4)
                    nc.gpsimd.partition_broadcast(rb[:, 0:1], rr[0:1, 0:1])
                    nc.gpsimd.partition_broadcast(a_b[:, off:off + nr],
                                                  arow[0:1, off:off + nr])
                    attc = spool.tile([128, HC], f32, name=f"attc_{gi}_{b}",
                                      tag="attc", bufs=2)
                    scr2 = wpool.tile([128, nr], f16, name=f"sc2_{gi}_{b}",
                                      tag=f"scr2_{side}", bufs=2)
                    for hc in range(HC):
                        nc.vector.scalar_tensor_tensor(
                            out=scr2[:, 0:nr],
                            in0=xt3[side][:, hc, g0 + off:g0 + off + nr],
                            scalar=1.0, in1=a_b[:, off:off + nr],
                            op0=Alu.mult, op1=Alu.mult,
                            accum_out=attc[:, hc:hc + 1])
                    nc.scalar.activation(
                        att_sb[side][:, b * HC:(b + 1) * HC], attc[:, 0:HC],
                        Act.Copy, scale=rb[:, 0:1])

            score7, tail = None, None
            for gi in range(len(groups)):
                score7, tail = emit_group(gi, score7, tail)
            score7()
            tail()

            for side in ("img", "dns"):
                nc.sync.dma_start(
                    out=out_d[side].rearrange("p b hc -> p (b hc)"),
                    in_=att_sb[side][:, :])
    nc.compile()
    return nc


def _get_nc():
    if "nc" not in _CACHE:
        _CACHE["nc"] = build_nc()
    return _CACHE["nc"]


def make_in_maps(inputs):
    dns = np.ascontiguousarray(np.asarray(inputs["dns_feature"], dtype=np.float32))
    img = np.ascontiguousarray(np.asarray(inputs["img_features"], dtype=np.float32))
    W_i1 = np.asarray(inputs["W_i1"], dtype=np.float32)
    W_d2 = np.asarray(inputs["W_d2"], dtype=np.float32)
    wB = np.asarray(inputs["w_att1"], dtype=np.float32)[H:]
    wD = np.asarray(inputs["w_att2"], dtype=np.float32)[H:]

    # W.T [h, o] -> [oc, hc, 128(h), 128(o)]
    def wt_pack(W):
        wt = W.T.reshape(HC, 128, OC, 128).transpose(2, 0, 1, 3)
        return np.ascontiguousarray(wt).astype(_BF16)

    wt_i1 = wt_pack(W_i1)
    wt_d2 = wt_pack(W_d2)
    wcol_b = np.ascontiguousarray(wB.reshape(OC, 128).T).astype(_BF16)
    wcol_d = np.ascontiguousarray(wD.reshape(OC, 128).T).astype(_BF16)

    in_maps = []
    for k in range(NCORES):
        sl = slice(k * BLOC, (k + 1) * BLOC)
        # [BLOC, rows, H] -> [H, BLOC*rows] -> [HC, 128, n]
        xt_d = dns[sl].reshape(BLOC * S, H).T.reshape(HC, 128, BLOC * S)
        xt_i = img[sl].reshape(BLOC * R, H).T.reshape(HC, 128, BLOC * R)
        in_maps.append({
            "xt_dns": np.ascontiguousarray(xt_d).astype(_BF16),
            "xt_img": np.ascontiguousarray(xt_i).astype(_BF16),
            "wt_i1": wt_i1,
            "wt_d2": wt_d2,
            "wcol_b": wcol_b,
            "wcol_d": wcol_d,
        })
    return in_maps


def kernel(**inputs):
    from concourse.bass_utils import run_bass_kernel_spmd

    nc = _get_nc()
    in_maps = make_in_maps(inputs)
    res = run_bass_kernel_spmd(nc, in_maps, list(range(NCORES))).results
    # device out: [128, BLOC, HC], element h of batch b at [h % 128, b, h//128]
    outs = {}
    for name in ("out_dns", "out_img"):
        per = [res[k][name].transpose(1, 2, 0).reshape(BLOC, H)
               for k in range(NCORES)]
        outs[name] = np.concatenate(per, axis=0)
    out_dns = np.ascontiguousarray(
        np.broadcast_to(outs["out_dns"][:, None, :], (B, S, H)))
    out_img = np.ascontiguousarray(
        np.broadcast_to(outs["out_img"][:, None, :], (B, S, H)))
    return out_dns, out_img
